# revision 41
# baseline (speedup 1.0000x reference)
"""CrossScaleAttention Trainium2 kernel: 8-core SPMD via bass/tile.

Sharding: core (s, py) = (core//2, core%2): sample s = core//2, output row
parity py. Each core computes full attention for its sample and the deconv
for its output-row parity. Host prepares small gather tensors (q_col, kpT,
ap taps — <0.1% of FLOPs); device does scores matmuls (fp16), softmax
(exp/Z/normalize, bf16 att) and the conv-transpose matmuls (bf16).

Schedule: double-buffered att stripes; the deconv matmuls of stripe s-1 are
interleaved into the ACT-bound softmax phase of stripe s so the PE never
idles waiting on exp.
"""
import sys, types
sys.path.insert(0, "/opt/trn_rl_repo")
import numpy as np
import ml_dtypes
from contextlib import ExitStack

# NTFF profile hook shim (image's antenv lacks axon_hooks)
try:
    import trn_agent_boot.trn_boot as _tb
    _hook = _tb._ntff_profile_via_ctypes('/opt/axon/libaxon_pjrt.so')
    _m = types.ModuleType("antenv.axon_hooks")
    _m.get_axon_ntff_profile_hook = lambda: _hook
    _m.set_axon_ntff_profile_hook = lambda h: None
    sys.modules["antenv.axon_hooks"] = _m
except Exception:
    pass

import concourse.bass as bass
import concourse.tile as tile
import concourse.mybir as mybir
from concourse import bacc
from concourse.bass_utils import run_bass_kernel_spmd

F32 = mybir.dt.float32
F32R = mybir.dt.float32r
F16 = mybir.dt.float16
BF16 = mybir.dt.bfloat16
AF = mybir.ActivationFunctionType

C, Cr, B, H, W, L = 64, 16, 4, 96, 96, 2304
NCH = 18           # l-chunks of 128
ST_A = 12          # a-rows (output row-pairs) per stripe
RWS = ST_A + 2     # att i-rows buffered per stripe
NST = 96 // ST_A   # stripes

last_exec_time_ns = None

_cache = {}


def _build_program():
    nc = bacc.Bacc("TRN2", target_bir_lowering=False, debug=False, num_devices=8)
    qA_d = nc.dram_tensor("qcolA", [72, H * W], F16, kind="ExternalInput").ap()
    qB_d = nc.dram_tensor("qcolB", [72, H * W], F16, kind="ExternalInput").ap()
    kA_d = nc.dram_tensor("kpTA", [72, L], F16, kind="ExternalInput").ap()
    kB_d = nc.dram_tensor("kpTB", [72, L], F16, kind="ExternalInput").ap()
    ap_d = nc.dram_tensor("ap", [128, NCH * 9 * 128], BF16, kind="ExternalInput").ap()
    oh_d = nc.dram_tensor("oh", [64, 96 * 192], F32, kind="ExternalOutput").ap()

    with tile.TileContext(nc) as tc:
        with ExitStack() as ctx:
            pm = ctx.enter_context(tc.tile_pool(name="main", bufs=1))
            pq = ctx.enter_context(tc.tile_pool(name="q", bufs=2))
            pob = ctx.enter_context(tc.tile_pool(name="ob", bufs=3))
            prz = ctx.enter_context(tc.tile_pool(name="rz", bufs=2))
            pps = ctx.enter_context(tc.tile_pool(name="ps", bufs=3, space="PSUM"))
            ppd = ctx.enter_context(tc.tile_pool(name="pd", bufs=3, space="PSUM"))
            ppz = ctx.enter_context(tc.tile_pool(name="pz", bufs=2, space="PSUM"))

            # persistent operands
            kA = pm.tile([72, L], F16, tag="kA")
            nc.sync.dma_start(kA[:], kA_d)
            kB = pm.tile([72, L], F16, tag="kB")
            nc.sync.dma_start(kB[:], kB_d)
            apall = pm.tile([128, NCH * 9 * 128], BF16, tag="apall")
            nc.sync.dma_start(apall[:], ap_d)
            # full [128,128] ones for Z: keeps the PE in full-array config and
            # leaves Z broadcast across all partitions (no separate bcast MM)
            o128 = pm.tile([128, 128], BF16, tag="o128")
            nc.vector.memset(o128[:], 1.0)

            # att stripe buffers (double-buffered), bf16, one big tile each:
            # layout per partition: [k(NCH), r(RWS), c(98)]; cols 0,97 are pad
            attb = []
            for h in range(2):
                t = pm.tile([128, NCH * RWS * 98], BF16, tag=f"att{h}")
                attb.append(t)

            def chunk_view(h, k):
                return attb[h][:, k * RWS * 98:(k + 1) * RWS * 98] \
                    .rearrange("p (r c) -> p r c", c=98)

            for h in range(2):
                for k in range(NCH):
                    v = chunk_view(h, k)
                    for pc in (0, 97):
                        nc.vector.memset(v[:, :, pc:pc + 1], 0.0)
                    # stripe-0 boundary row (i=-1) zero
                    nc.vector.memset(v[:, 0:1, :], 0.0)

            # deconv MM emitters: one group = 162 accumulating MMs over G a-rows
            # (k outer so the normalize->deconv dependency ramps one chunk at
            # a time instead of needing 9 chunks normalized up front)
            def deconv_mms(h, g0, G):
                """Operand list for the 162 matmuls of a G-a-row deconv group."""
                out = []
                for k in range(NCH):
                    v = chunk_view(h, k)
                    for n in range(3):
                        for m in range(3):
                            nm = n * 3 + m
                            r0 = g0 + 2 - n
                            off = (k * 9 + nm) * 128
                            rhs = v[:, r0:r0 + G, 2 - m:98 - m]
                            out.append((apall[:, off:off + 128], rhs))
                return out

            # state of the pending (previous-stripe) deconv
            pending = None   # (h, arow, G, mm list, next index, dps tile)

            def emit_deconv_slice(cnt):
                nonlocal pending
                while cnt > 0:
                    if pending is None:
                        if not deconv_queue:
                            return
                        start_deconv(*deconv_queue.pop(0))
                    h, arow, G, mms, idx, dps = pending
                    end = min(idx + cnt, len(mms))
                    for i in range(idx, end):
                        lw, rhs = mms[i]
                        nc.tensor.matmul(dps[:, :G * 96], lw, rhs,
                                         start=(i == 0), stop=(i == len(mms) - 1))
                    cnt -= end - idx
                    if end == len(mms):
                        ob = pob.tile([128, 480], F32, tag="ob")
                        nc.scalar.activation(ob[:, :G * 96], dps[:, :G * 96], AF.Copy)
                        oap = oh_d.rearrange("p (y x) -> p y x", x=192)
                        nc.sync.dma_start(oap[:, arow:arow + G, 0:96],
                                          ob[0:64, :G * 96].rearrange("p (r c) -> p r c", c=96))
                        nc.sync.dma_start(oap[:, arow:arow + G, 96:192],
                                          ob[64:128, :G * 96].rearrange("p (r c) -> p r c", c=96))
                        pending = None
                    else:
                        pending = (h, arow, G, mms, end, dps)

            def start_deconv(h, arow, g0, G):
                nonlocal pending
                assert pending is None
                dps = ppd.tile([128, 480], F32, tag="dps")
                pending = (h, arow, G, deconv_mms(h, g0, G), 0, dps)

            deconv_queue = []   # (h, arow, g0, G) groups not yet started

            for st in range(NST):
                h = st % 2
                a0 = st * ST_A
                r_lo = 1 if st == 0 else 0
                r_hi = RWS - 1 if st == NST - 1 else RWS
                i_lo = a0 - 1 + r_lo
                nrows = r_hi - r_lo
                qA = pq.tile([72, RWS * 96], F16, tag="qA")
                qB = pq.tile([72, RWS * 96], F16, tag="qB")
                nc.sync.dma_start(qA[:, r_lo * 96: (r_lo + nrows) * 96],
                                  qA_d[:, i_lo * 96: (i_lo + nrows) * 96])
                nc.sync.dma_start(qB[:, r_lo * 96: (r_lo + nrows) * 96],
                                  qB_d[:, i_lo * 96: (i_lo + nrows) * 96])
                if st == NST - 1:
                    # boundary row (i=96) zero, this buffer's last row
                    for k in range(NCH):
                        nc.vector.memset(chunk_view(h, k)[:, RWS - 1:RWS, :], 0.0)

                groups = []
                r = r_lo
                while r < r_hi:
                    sz = min(5, r_hi - r)
                    groups.append((r, sz))
                    r += sz
                # this stripe's own deconv groups, released once the att rows
                # they read have had their normalize emitted (boundary rows
                # beyond r_hi-1 are memset zeros and never normalized)
                own = []
                g0 = 0
                while g0 < ST_A:
                    G = min(5, ST_A - g0)
                    own.append((h, a0 + g0, g0, G, min(g0 + G + 1, r_hi - 1)))
                    g0 += G

                def release(norm_end, own=own):
                    while own and own[0][4] < norm_end:
                        deconv_queue.append(own.pop(0)[:4])

                for (rg, sz) in groups:
                    N = sz * 96
                    zps = ppz.tile([128, 512], F32, tag="zps")
                    for k in range(NCH):
                        ps = pps.tile([128, 512], F32, tag="ps")
                        nc.tensor.matmul(ps[:, :N], kA[:, 128 * k:128 * (k + 1)],
                                         qA[:, rg * 96: rg * 96 + N],
                                         start=True, stop=False)
                        nc.tensor.matmul(ps[:, :N], kB[:, 128 * k:128 * (k + 1)],
                                         qB[:, rg * 96: rg * 96 + N],
                                         start=False, stop=True)
                        # fill PE with already-released deconv while ACT exps
                        emit_deconv_slice(9)
                        # exp(s) from psum -> att rows (scale folded into kpT)
                        dst = chunk_view(h, k)[:, rg:rg + sz, 1:97]
                        nc.scalar.activation(dst, ps[:, :N].rearrange("p (r c) -> p r c", c=96),
                                             AF.Exp)
                        # Z accumulation, pre-broadcast to all 128 partitions
                        nc.tensor.matmul(zps[:, :N], o128[:], dst,
                                         start=(k == 0), stop=(k == NCH - 1))

                    # normalize (all DVE): 1/Z straight to bf16, then scale att
                    bsb = prz.tile([128, 512], BF16, tag="bsb")
                    with nc.allow_low_precision(reason="1/Z in bf16 scales att"):
                        nc.vector.reciprocal(bsb[:, :N], zps[:, :N])
                    for k in range(NCH):
                        a_ap = chunk_view(h, k)[:, rg:rg + sz, 1:97]
                        nc.vector.tensor_mul(a_ap, a_ap,
                                             bsb[:, :N].rearrange("p (r c) -> p r c", c=96))
                    release(rg + sz)

                release(RWS)
                assert not own
                # drain any unfinished pending deconv before stripe ends?
                # no — let it continue into the next stripe's blocks.

            # drain remaining deconv groups
            emit_deconv_slice(10 ** 9)
    nc.compile()
    return nc


def _prelu(z, a):
    return np.where(z >= 0, z, a * z)


def _host_prep(x, wa, ba, aa, w1, b1, a1, w2, b2, a2):
    """Per-sample gather prep (numpy, validated vs reference)."""
    f32 = np.float32
    per_core = []
    waT_aug = (np.concatenate([wa.T, ba[None, :]], 0) / 6.0).astype(f32)
    w1T_aug = np.concatenate([w1.T, b1[None, :]], 0).astype(f32)
    w2T_aug = np.concatenate([w2.T / 4.0, b2[None, :]], 0).astype(f32)
    aav, a1v, a2v = float(aa[0]), float(a1[0]), float(a2[0])
    for s in range(B):
        xs = np.asarray(x[s], f32)
        xq_aug = np.concatenate([xs.reshape(64, -1), np.ones((1, H * W), f32)], 0)
        asmT = _prelu(xq_aug.T @ waT_aug, aav)
        qT = _prelu(xq_aug.T @ w1T_aug, a1v)
        x3 = xs.reshape(64, 96, 96)
        t1 = x3[:, :, 0::2] + x3[:, :, 1::2]
        xd = t1[:, 0::2, :] + t1[:, 1::2, :]
        xd_aug = np.concatenate([xd.reshape(64, -1), np.ones((1, 48 * 48), f32)], 0)
        kfT = _prelu(xd_aug.T @ w2T_aug, a2v)

        kf = kfT.T.reshape(Cr, 48, 48)
        kpT = np.zeros((144, L), f32)
        for t, (dy, dx) in enumerate([(a, b) for a in range(3) for b in range(3)]):
            ly_lo, ly_hi = max(0, 1 - dy), min(48, 49 - dy)
            lx_lo, lx_hi = max(0, 1 - dx), min(48, 49 - dx)
            blk = kf[:, ly_lo + dy - 1:ly_hi + dy - 1, lx_lo + dx - 1:lx_hi + dx - 1]
            dst = kpT[16 * t:16 * t + 16].reshape(Cr, 48, 48)
            dst[:, ly_lo:ly_hi, lx_lo:lx_hi] = blk
        nrm = np.sqrt((kpT ** 2).sum(0))
        rnorm10 = (10.0 / np.maximum(nrm, 1e-4)).astype(f32)
        # fold the softmax scale / norm into kpT: scores psum = 10*s/norm
        kpT = kpT * rnorm10[None, :]

        q3 = qT.T.reshape(Cr, 96, 96)
        q_col = np.zeros((144, 96, 96), f32)
        for t, (dy, dx) in enumerate([(a, b) for a in range(3) for b in range(3)]):
            y_lo, y_hi = max(0, 1 - dy), min(96, 97 - dy)
            x_lo, x_hi = max(0, 1 - dx), min(96, 97 - dx)
            q_col[16 * t:16 * t + 16, y_lo:y_hi, x_lo:x_hi] = \
                q3[:, y_lo + dy - 1:y_hi + dy - 1, x_lo + dx - 1:x_hi + dx - 1]
        q_col = q_col.reshape(144, H * W)

        asm3 = asmT.T.reshape(64, 96, 96)
        for py in (0, 1):
            ap_t = np.zeros((3, 3, L, 128), f32)
            for n in range(3):
                u = py + 2 * n
                for m in range(3):
                    for half, v in ((0, 2 * m), (1, 2 * m + 1)):
                        ly_lo = max(0, (3 - u) // 2)
                        ly_hi = min(48, (99 - u) // 2)
                        lx_lo = max(0, (3 - v) // 2)
                        lx_hi = min(48, (97 - v) // 2 + 1)
                        Y0, X0 = 2 * ly_lo + u - 2, 2 * lx_lo + v - 2
                        blk = asm3[:, Y0:Y0 + 2 * (ly_hi - ly_lo):2,
                                   X0:X0 + 2 * (lx_hi - lx_lo):2]
                        dst = ap_t[n, m, :, 64 * half:64 * half + 64].reshape(48, 48, 64)
                        dst[ly_lo:ly_hi, lx_lo:lx_hi, :] = blk.transpose(1, 2, 0)
            # device ap layout: [p(128), k(18), nm(9), c(128)] for one-DMA load
            ap2 = np.ascontiguousarray(
                ap_t.reshape(9, NCH, 128, 128).transpose(2, 1, 0, 3)
            ).reshape(128, NCH * 9 * 128).astype(ml_dtypes.bfloat16)
            per_core.append({
                "qcolA": np.ascontiguousarray(q_col[:72]).astype(np.float16),
                "qcolB": np.ascontiguousarray(q_col[72:144]).astype(np.float16),
                "kpTA": np.ascontiguousarray(kpT[:72]).astype(np.float16),
                "kpTB": np.ascontiguousarray(kpT[72:144]).astype(np.float16),
                "ap": ap2,
            })
    return per_core


def kernel(x, wa, ba, aa, w1, b1, a1, w2, b2, a2):
    global last_exec_time_ns
    if "nc" not in _cache:
        _cache["nc"] = _build_program()
    nc = _cache["nc"]
    in_maps = _host_prep(np.asarray(x, np.float32), np.asarray(wa), np.asarray(ba),
                         np.asarray(aa), np.asarray(w1), np.asarray(b1),
                         np.asarray(a1), np.asarray(w2), np.asarray(b2),
                         np.asarray(a2))
    import os
    trace = bool(int(os.environ.get("KERNEL_TRACE", "0")))
    res = run_bass_kernel_spmd(nc, in_maps, core_ids=list(range(8)), trace=trace)
    last_exec_time_ns = res.exec_time_ns
    out = np.zeros((B, C, 192, 192), np.float32)
    for core in range(8):
        s, py = core // 2, core % 2
        r = res.results[core]["oh"].reshape(64, 96, 2, 96)
        out[s, :, py::2, 0::2] = r[:, :, 0, :]
        out[s, :, py::2, 1::2] = r[:, :, 1, :]
    return out


# revision 43
# speedup vs baseline: 1.0365x; 1.0365x over previous
"""CrossScaleAttention Trainium2 kernel: 8-core SPMD via bass/tile.

Sharding: core (s, py) = (core//2, core%2): sample s = core//2, output row
parity py. Each core computes full attention for its sample and the deconv
for its output-row parity. Host prepares small gather tensors (q_col, kpT,
ap taps — <0.1% of FLOPs); device does scores matmuls (fp16), softmax
(exp/Z/normalize, bf16 att) and the conv-transpose matmuls (bf16).

Schedule: double-buffered att stripes; the deconv matmuls of stripe s-1 are
interleaved into the ACT-bound softmax phase of stripe s so the PE never
idles waiting on exp.
"""
import sys, types
sys.path.insert(0, "/opt/trn_rl_repo")
import numpy as np
import ml_dtypes
from contextlib import ExitStack

# NTFF profile hook shim (image's antenv lacks axon_hooks)
try:
    import trn_agent_boot.trn_boot as _tb
    _hook = _tb._ntff_profile_via_ctypes('/opt/axon/libaxon_pjrt.so')
    _m = types.ModuleType("antenv.axon_hooks")
    _m.get_axon_ntff_profile_hook = lambda: _hook
    _m.set_axon_ntff_profile_hook = lambda h: None
    sys.modules["antenv.axon_hooks"] = _m
except Exception:
    pass

import concourse.bass as bass
import concourse.tile as tile
import concourse.mybir as mybir
from concourse import bacc
from concourse.bass_utils import run_bass_kernel_spmd

F32 = mybir.dt.float32
F32R = mybir.dt.float32r
F16 = mybir.dt.float16
BF16 = mybir.dt.bfloat16
AF = mybir.ActivationFunctionType

C, Cr, B, H, W, L = 64, 16, 4, 96, 96, 2304
NCH = 18           # l-chunks of 128
ST_A = 12          # a-rows (output row-pairs) per stripe
RWS = ST_A + 2     # att i-rows buffered per stripe
NST = 96 // ST_A   # stripes

last_exec_time_ns = None

_cache = {}


def _build_program():
    nc = bacc.Bacc("TRN2", target_bir_lowering=False, debug=False, num_devices=8)
    qA_d = nc.dram_tensor("qcolA", [72, H * W], F16, kind="ExternalInput").ap()
    qB_d = nc.dram_tensor("qcolB", [72, H * W], F16, kind="ExternalInput").ap()
    kA_d = nc.dram_tensor("kpTA", [72, L], F16, kind="ExternalInput").ap()
    kB_d = nc.dram_tensor("kpTB", [72, L], F16, kind="ExternalInput").ap()
    ap_d = nc.dram_tensor("ap", [128, NCH * 9 * 128], BF16, kind="ExternalInput").ap()
    oh_d = nc.dram_tensor("oh", [64, 96 * 192], F32, kind="ExternalOutput").ap()

    with tile.TileContext(nc) as tc:
        with ExitStack() as ctx:
            pm = ctx.enter_context(tc.tile_pool(name="main", bufs=1))
            pq = ctx.enter_context(tc.tile_pool(name="q", bufs=2))
            pob = ctx.enter_context(tc.tile_pool(name="ob", bufs=3))
            prz = ctx.enter_context(tc.tile_pool(name="rz", bufs=2))
            pps = ctx.enter_context(tc.tile_pool(name="ps", bufs=3, space="PSUM"))
            ppd = ctx.enter_context(tc.tile_pool(name="pd", bufs=3, space="PSUM"))
            ppz = ctx.enter_context(tc.tile_pool(name="pz", bufs=2, space="PSUM"))

            # persistent operands
            kA = pm.tile([72, L], F16, tag="kA")
            nc.sync.dma_start(kA[:], kA_d)
            kB = pm.tile([72, L], F16, tag="kB")
            nc.sync.dma_start(kB[:], kB_d)
            apall = pm.tile([128, NCH * 9 * 128], BF16, tag="apall")
            nc.sync.dma_start(apall[:], ap_d)
            # full [128,128] ones for Z: keeps the PE in full-array config and
            # leaves Z broadcast across all partitions (no separate bcast MM)
            o128 = pm.tile([128, 128], BF16, tag="o128")
            nc.vector.memset(o128[:], 1.0)

            # att stripe buffers (double-buffered), bf16, one big tile each:
            # layout per partition: [k(NCH), r(RWS), c(98)]; cols 0,97 are pad
            attb = []
            for h in range(2):
                t = pm.tile([128, NCH * RWS * 98], BF16, tag=f"att{h}")
                attb.append(t)

            def chunk_view(h, k):
                return attb[h][:, k * RWS * 98:(k + 1) * RWS * 98] \
                    .rearrange("p (r c) -> p r c", c=98)

            for h in range(2):
                for k in range(NCH):
                    v = chunk_view(h, k)
                    for pc in (0, 97):
                        nc.vector.memset(v[:, :, pc:pc + 1], 0.0)
                    # stripe-0 boundary row (i=-1) zero
                    nc.vector.memset(v[:, 0:1, :], 0.0)

            # deconv MM emitters: one group = 162 accumulating MMs over G a-rows
            # (k outer so the normalize->deconv dependency ramps one chunk at
            # a time instead of needing 9 chunks normalized up front)
            def deconv_mms(h, g0, G):
                """Operand list for the 162 matmuls of a G-a-row deconv group."""
                out = []
                for k in range(NCH):
                    v = chunk_view(h, k)
                    for n in range(3):
                        for m in range(3):
                            nm = n * 3 + m
                            r0 = g0 + 2 - n
                            off = (k * 9 + nm) * 128
                            rhs = v[:, r0:r0 + G, 2 - m:98 - m]
                            out.append((apall[:, off:off + 128], rhs))
                return out

            # state of the pending (previous-stripe) deconv
            pending = None   # (h, arow, G, mm list, next index, dps tile)
            deferred = None  # deferred normalize tail of the previous group

            def emit_deconv_slice(cnt):
                nonlocal pending
                while cnt > 0:
                    if pending is None:
                        if not deconv_queue:
                            return
                        start_deconv(*deconv_queue.pop(0))
                    h, arow, G, mms, idx, dps = pending
                    end = min(idx + cnt, len(mms))
                    for i in range(idx, end):
                        lw, rhs = mms[i]
                        nc.tensor.matmul(dps[:, :G * 96], lw, rhs,
                                         start=(i == 0), stop=(i == len(mms) - 1))
                    cnt -= end - idx
                    if end == len(mms):
                        ob = pob.tile([128, 480], F32, tag="ob")
                        nc.scalar.activation(ob[:, :G * 96], dps[:, :G * 96], AF.Copy)
                        oap = oh_d.rearrange("p (y x) -> p y x", x=192)
                        nc.sync.dma_start(oap[:, arow:arow + G, 0:96],
                                          ob[0:64, :G * 96].rearrange("p (r c) -> p r c", c=96))
                        nc.sync.dma_start(oap[:, arow:arow + G, 96:192],
                                          ob[64:128, :G * 96].rearrange("p (r c) -> p r c", c=96))
                        pending = None
                    else:
                        pending = (h, arow, G, mms, end, dps)

            def start_deconv(h, arow, g0, G):
                nonlocal pending
                assert pending is None
                dps = ppd.tile([128, 480], F32, tag="dps")
                pending = (h, arow, G, deconv_mms(h, g0, G), 0, dps)

            deconv_queue = []   # (h, arow, g0, G) groups not yet started

            for st in range(NST):
                h = st % 2
                a0 = st * ST_A
                r_lo = 1 if st == 0 else 0
                r_hi = RWS - 1 if st == NST - 1 else RWS
                i_lo = a0 - 1 + r_lo
                nrows = r_hi - r_lo
                qA = pq.tile([72, RWS * 96], F16, tag="qA")
                qB = pq.tile([72, RWS * 96], F16, tag="qB")
                nc.sync.dma_start(qA[:, r_lo * 96: (r_lo + nrows) * 96],
                                  qA_d[:, i_lo * 96: (i_lo + nrows) * 96])
                nc.sync.dma_start(qB[:, r_lo * 96: (r_lo + nrows) * 96],
                                  qB_d[:, i_lo * 96: (i_lo + nrows) * 96])
                if st == NST - 1:
                    # boundary row (i=96) zero, this buffer's last row
                    for k in range(NCH):
                        nc.vector.memset(chunk_view(h, k)[:, RWS - 1:RWS, :], 0.0)

                groups = []
                r = r_lo
                while r < r_hi:
                    sz = min(5, r_hi - r)
                    groups.append((r, sz))
                    r += sz
                for (rg, sz) in groups:
                    N = sz * 96
                    zps = ppz.tile([128, 512], F32, tag="zps")
                    prev_dst = None
                    for k in range(NCH + 1):
                        if k < NCH:
                            ps = pps.tile([128, 512], F32, tag="ps")
                            nc.tensor.matmul(ps[:, :N], kA[:, 128 * k:128 * (k + 1)],
                                             qA[:, rg * 96: rg * 96 + N],
                                             start=True, stop=False)
                        if prev_dst is not None:
                            # Z accumulation, pre-broadcast to all 128 partitions;
                            # placed between the sA/sB pair so the pair pipelines
                            # even when no deconv filler is available (stripe 0)
                            nc.tensor.matmul(zps[:, :N], o128[:], prev_dst,
                                             start=(k == 1), stop=(k == NCH))
                        if k == NCH:
                            break
                        nc.tensor.matmul(ps[:, :N], kB[:, 128 * k:128 * (k + 1)],
                                         qB[:, rg * 96: rg * 96 + N],
                                         start=False, stop=True)
                        # fill PE with previous-stripe deconv while ACT exps
                        emit_deconv_slice(9)
                        if k == 3 and deferred is not None:
                            deferred()
                            deferred = None
                        # exp(s) from psum -> att rows (scale folded into kpT)
                        dst = chunk_view(h, k)[:, rg:rg + sz, 1:97]
                        nc.scalar.activation(dst, ps[:, :N].rearrange("p (r c) -> p r c", c=96),
                                             AF.Exp)
                        prev_dst = dst

                    def tail(h=h, rg=rg, sz=sz, N=N, zps=zps):
                        # normalize: 1/Z straight to bf16, then scale att
                        bsb = prz.tile([128, 512], BF16, tag="bsb")
                        with nc.allow_low_precision(reason="1/Z in bf16 scales att"):
                            nc.vector.reciprocal(bsb[:, :N], zps[:, :N])
                        for k in range(NCH):
                            a_ap = chunk_view(h, k)[:, rg:rg + sz, 1:97]
                            nc.vector.tensor_mul(a_ap, a_ap,
                                                 bsb[:, :N].rearrange("p (r c) -> p r c", c=96))
                    deferred = tail

                # queue this stripe's deconv groups (run during next stripe)
                g0 = 0
                while g0 < ST_A:
                    G = min(5, ST_A - g0)
                    deconv_queue.append((h, a0 + g0, g0, G))
                    g0 += G
                # drain any unfinished pending deconv before stripe ends?
                # no — let it continue into the next stripe's blocks.

            # flush the last normalize tail, then drain remaining deconv groups
            if deferred is not None:
                deferred()
                deferred = None
            emit_deconv_slice(10 ** 9)
    nc.compile()
    return nc


def _prelu(z, a):
    return np.where(z >= 0, z, a * z)


def _host_prep(x, wa, ba, aa, w1, b1, a1, w2, b2, a2):
    """Per-sample gather prep (numpy, validated vs reference)."""
    f32 = np.float32
    per_core = []
    waT_aug = (np.concatenate([wa.T, ba[None, :]], 0) / 6.0).astype(f32)
    w1T_aug = np.concatenate([w1.T, b1[None, :]], 0).astype(f32)
    w2T_aug = np.concatenate([w2.T / 4.0, b2[None, :]], 0).astype(f32)
    aav, a1v, a2v = float(aa[0]), float(a1[0]), float(a2[0])
    for s in range(B):
        xs = np.asarray(x[s], f32)
        xq_aug = np.concatenate([xs.reshape(64, -1), np.ones((1, H * W), f32)], 0)
        asmT = _prelu(xq_aug.T @ waT_aug, aav)
        qT = _prelu(xq_aug.T @ w1T_aug, a1v)
        x3 = xs.reshape(64, 96, 96)
        t1 = x3[:, :, 0::2] + x3[:, :, 1::2]
        xd = t1[:, 0::2, :] + t1[:, 1::2, :]
        xd_aug = np.concatenate([xd.reshape(64, -1), np.ones((1, 48 * 48), f32)], 0)
        kfT = _prelu(xd_aug.T @ w2T_aug, a2v)

        kf = kfT.T.reshape(Cr, 48, 48)
        kpT = np.zeros((144, L), f32)
        for t, (dy, dx) in enumerate([(a, b) for a in range(3) for b in range(3)]):
            ly_lo, ly_hi = max(0, 1 - dy), min(48, 49 - dy)
            lx_lo, lx_hi = max(0, 1 - dx), min(48, 49 - dx)
            blk = kf[:, ly_lo + dy - 1:ly_hi + dy - 1, lx_lo + dx - 1:lx_hi + dx - 1]
            dst = kpT[16 * t:16 * t + 16].reshape(Cr, 48, 48)
            dst[:, ly_lo:ly_hi, lx_lo:lx_hi] = blk
        nrm = np.sqrt((kpT ** 2).sum(0))
        rnorm10 = (10.0 / np.maximum(nrm, 1e-4)).astype(f32)
        # fold the softmax scale / norm into kpT: scores psum = 10*s/norm
        kpT = kpT * rnorm10[None, :]

        q3 = qT.T.reshape(Cr, 96, 96)
        q_col = np.zeros((144, 96, 96), f32)
        for t, (dy, dx) in enumerate([(a, b) for a in range(3) for b in range(3)]):
            y_lo, y_hi = max(0, 1 - dy), min(96, 97 - dy)
            x_lo, x_hi = max(0, 1 - dx), min(96, 97 - dx)
            q_col[16 * t:16 * t + 16, y_lo:y_hi, x_lo:x_hi] = \
                q3[:, y_lo + dy - 1:y_hi + dy - 1, x_lo + dx - 1:x_hi + dx - 1]
        q_col = q_col.reshape(144, H * W)

        asm3 = asmT.T.reshape(64, 96, 96)
        for py in (0, 1):
            ap_t = np.zeros((3, 3, L, 128), f32)
            for n in range(3):
                u = py + 2 * n
                for m in range(3):
                    for half, v in ((0, 2 * m), (1, 2 * m + 1)):
                        ly_lo = max(0, (3 - u) // 2)
                        ly_hi = min(48, (99 - u) // 2)
                        lx_lo = max(0, (3 - v) // 2)
                        lx_hi = min(48, (97 - v) // 2 + 1)
                        Y0, X0 = 2 * ly_lo + u - 2, 2 * lx_lo + v - 2
                        blk = asm3[:, Y0:Y0 + 2 * (ly_hi - ly_lo):2,
                                   X0:X0 + 2 * (lx_hi - lx_lo):2]
                        dst = ap_t[n, m, :, 64 * half:64 * half + 64].reshape(48, 48, 64)
                        dst[ly_lo:ly_hi, lx_lo:lx_hi, :] = blk.transpose(1, 2, 0)
            # device ap layout: [p(128), k(18), nm(9), c(128)] for one-DMA load
            ap2 = np.ascontiguousarray(
                ap_t.reshape(9, NCH, 128, 128).transpose(2, 1, 0, 3)
            ).reshape(128, NCH * 9 * 128).astype(ml_dtypes.bfloat16)
            per_core.append({
                "qcolA": np.ascontiguousarray(q_col[:72]).astype(np.float16),
                "qcolB": np.ascontiguousarray(q_col[72:144]).astype(np.float16),
                "kpTA": np.ascontiguousarray(kpT[:72]).astype(np.float16),
                "kpTB": np.ascontiguousarray(kpT[72:144]).astype(np.float16),
                "ap": ap2,
            })
    return per_core


def kernel(x, wa, ba, aa, w1, b1, a1, w2, b2, a2):
    global last_exec_time_ns
    if "nc" not in _cache:
        _cache["nc"] = _build_program()
    nc = _cache["nc"]
    in_maps = _host_prep(np.asarray(x, np.float32), np.asarray(wa), np.asarray(ba),
                         np.asarray(aa), np.asarray(w1), np.asarray(b1),
                         np.asarray(a1), np.asarray(w2), np.asarray(b2),
                         np.asarray(a2))
    import os
    trace = bool(int(os.environ.get("KERNEL_TRACE", "0")))
    res = run_bass_kernel_spmd(nc, in_maps, core_ids=list(range(8)), trace=trace)
    last_exec_time_ns = res.exec_time_ns
    out = np.zeros((B, C, 192, 192), np.float32)
    for core in range(8):
        s, py = core // 2, core % 2
        r = res.results[core]["oh"].reshape(64, 96, 2, 96)
        out[s, :, py::2, 0::2] = r[:, :, 0, :]
        out[s, :, py::2, 1::2] = r[:, :, 1, :]
    return out


# revision 45
# speedup vs baseline: 1.0570x; 1.0197x over previous
"""CrossScaleAttention Trainium2 kernel: 8-core SPMD via bass/tile.

Sharding: core (s, py) = (core//2, core%2): sample s = core//2, output row
parity py. Each core computes full attention for its sample and the deconv
for its output-row parity. Host prepares small gather tensors (q_col, kpT,
ap taps — <0.1% of FLOPs); device does scores matmuls (fp16), softmax
(exp/Z/normalize, bf16 att) and the conv-transpose matmuls (bf16).

Schedule: double-buffered att stripes; the deconv matmuls of stripe s-1 are
interleaved into the ACT-bound softmax phase of stripe s so the PE never
idles waiting on exp.
"""
import sys, types
sys.path.insert(0, "/opt/trn_rl_repo")
import numpy as np
import ml_dtypes
from contextlib import ExitStack

# NTFF profile hook shim (image's antenv lacks axon_hooks)
try:
    import trn_agent_boot.trn_boot as _tb
    _hook = _tb._ntff_profile_via_ctypes('/opt/axon/libaxon_pjrt.so')
    _m = types.ModuleType("antenv.axon_hooks")
    _m.get_axon_ntff_profile_hook = lambda: _hook
    _m.set_axon_ntff_profile_hook = lambda h: None
    sys.modules["antenv.axon_hooks"] = _m
except Exception:
    pass

import concourse.bass as bass
import concourse.tile as tile
import concourse.mybir as mybir
from concourse import bacc
from concourse.bass_utils import run_bass_kernel_spmd

F32 = mybir.dt.float32
F32R = mybir.dt.float32r
F16 = mybir.dt.float16
BF16 = mybir.dt.bfloat16
AF = mybir.ActivationFunctionType

C, Cr, B, H, W, L = 64, 16, 4, 96, 96, 2304
NCH = 18           # l-chunks of 128
ST_A = 12          # a-rows (output row-pairs) per stripe
RWS = ST_A + 2     # att i-rows buffered per stripe
NST = 96 // ST_A   # stripes

last_exec_time_ns = None

_cache = {}


def _build_program():
    nc = bacc.Bacc("TRN2", target_bir_lowering=False, debug=False, num_devices=8)
    qA_d = nc.dram_tensor("qcolA", [72, H * W], F16, kind="ExternalInput").ap()
    qB_d = nc.dram_tensor("qcolB", [72, H * W], F16, kind="ExternalInput").ap()
    kA_d = nc.dram_tensor("kpTA", [72, L], F16, kind="ExternalInput").ap()
    kB_d = nc.dram_tensor("kpTB", [72, L], F16, kind="ExternalInput").ap()
    ap_d = nc.dram_tensor("ap", [128, NCH * 9 * 128], BF16, kind="ExternalInput").ap()
    oh_d = nc.dram_tensor("oh", [64, 96 * 192], F32, kind="ExternalOutput").ap()

    with tile.TileContext(nc) as tc:
        with ExitStack() as ctx:
            pm = ctx.enter_context(tc.tile_pool(name="main", bufs=1))
            pq = ctx.enter_context(tc.tile_pool(name="q", bufs=2))
            pob = ctx.enter_context(tc.tile_pool(name="ob", bufs=3))
            prz = ctx.enter_context(tc.tile_pool(name="rz", bufs=2))
            pps = ctx.enter_context(tc.tile_pool(name="ps", bufs=3, space="PSUM"))
            ppd = ctx.enter_context(tc.tile_pool(name="pd", bufs=3, space="PSUM"))
            ppz = ctx.enter_context(tc.tile_pool(name="pz", bufs=2, space="PSUM"))

            # persistent operands
            kA = pm.tile([72, L], F16, tag="kA")
            nc.sync.dma_start(kA[:], kA_d)
            kB = pm.tile([72, L], F16, tag="kB")
            nc.sync.dma_start(kB[:], kB_d)
            # apall's 5.3MB DMA is deferred until after stripe 0's q loads:
            # HWDGE DMAs are FIFO per queue, and the first scores matmuls
            # only need q/k — the deconv weights aren't read until stripe 1
            apall = pm.tile([128, NCH * 9 * 128], BF16, tag="apall")
            # full [128,128] ones for Z: keeps the PE in full-array config and
            # leaves Z broadcast across all partitions (no separate bcast MM)
            o128 = pm.tile([128, 128], BF16, tag="o128")
            nc.vector.memset(o128[:], 1.0)

            # att stripe buffers (double-buffered), bf16, one big tile each:
            # layout per partition: [k(NCH), r(RWS), c(98)]; cols 0,97 are pad
            attb = []
            for h in range(2):
                t = pm.tile([128, NCH * RWS * 98], BF16, tag=f"att{h}")
                attb.append(t)

            def chunk_view(h, k):
                return attb[h][:, k * RWS * 98:(k + 1) * RWS * 98] \
                    .rearrange("p (r c) -> p r c", c=98)

            for h in range(2):
                for k in range(NCH):
                    v = chunk_view(h, k)
                    for pc in (0, 97):
                        nc.vector.memset(v[:, :, pc:pc + 1], 0.0)
                    # stripe-0 boundary row (i=-1) zero
                    nc.vector.memset(v[:, 0:1, :], 0.0)

            # deconv MM emitters: one group = 162 accumulating MMs over G a-rows
            # (k outer so the normalize->deconv dependency ramps one chunk at
            # a time instead of needing 9 chunks normalized up front)
            def deconv_mms(h, g0, G):
                """Operand list for the 162 matmuls of a G-a-row deconv group."""
                out = []
                for k in range(NCH):
                    v = chunk_view(h, k)
                    for n in range(3):
                        for m in range(3):
                            nm = n * 3 + m
                            r0 = g0 + 2 - n
                            off = (k * 9 + nm) * 128
                            rhs = v[:, r0:r0 + G, 2 - m:98 - m]
                            out.append((apall[:, off:off + 128], rhs))
                return out

            # state of the pending (previous-stripe) deconv
            pending = None   # (h, arow, G, mm list, next index, dps tile)
            deferred = None  # deferred normalize tail of the previous group

            def emit_deconv_slice(cnt):
                nonlocal pending
                while cnt > 0:
                    if pending is None:
                        if not deconv_queue:
                            return
                        start_deconv(*deconv_queue.pop(0))
                    h, arow, G, mms, idx, dps = pending
                    end = min(idx + cnt, len(mms))
                    for i in range(idx, end):
                        lw, rhs = mms[i]
                        nc.tensor.matmul(dps[:, :G * 96], lw, rhs,
                                         start=(i == 0), stop=(i == len(mms) - 1))
                    cnt -= end - idx
                    if end == len(mms):
                        ob = pob.tile([128, 480], F32, tag="ob")
                        nc.scalar.activation(ob[:, :G * 96], dps[:, :G * 96], AF.Copy)
                        oap = oh_d.rearrange("p (y x) -> p y x", x=192)
                        nc.sync.dma_start(oap[:, arow:arow + G, 0:96],
                                          ob[0:64, :G * 96].rearrange("p (r c) -> p r c", c=96))
                        nc.sync.dma_start(oap[:, arow:arow + G, 96:192],
                                          ob[64:128, :G * 96].rearrange("p (r c) -> p r c", c=96))
                        pending = None
                    else:
                        pending = (h, arow, G, mms, end, dps)

            def start_deconv(h, arow, g0, G):
                nonlocal pending
                assert pending is None
                dps = ppd.tile([128, 480], F32, tag="dps")
                pending = (h, arow, G, deconv_mms(h, g0, G), 0, dps)

            deconv_queue = []   # (h, arow, g0, G) groups not yet started

            for st in range(NST):
                h = st % 2
                a0 = st * ST_A
                r_lo = 1 if st == 0 else 0
                r_hi = RWS - 1 if st == NST - 1 else RWS
                i_lo = a0 - 1 + r_lo
                nrows = r_hi - r_lo
                qA = pq.tile([72, RWS * 96], F16, tag="qA")
                qB = pq.tile([72, RWS * 96], F16, tag="qB")
                nc.sync.dma_start(qA[:, r_lo * 96: (r_lo + nrows) * 96],
                                  qA_d[:, i_lo * 96: (i_lo + nrows) * 96])
                nc.sync.dma_start(qB[:, r_lo * 96: (r_lo + nrows) * 96],
                                  qB_d[:, i_lo * 96: (i_lo + nrows) * 96])
                if st == 0:
                    nc.sync.dma_start(apall[:], ap_d)
                if st == NST - 1:
                    # boundary row (i=96) zero, this buffer's last row
                    for k in range(NCH):
                        nc.vector.memset(chunk_view(h, k)[:, RWS - 1:RWS, :], 0.0)

                groups = []
                r = r_lo
                while r < r_hi:
                    sz = min(5, r_hi - r)
                    groups.append((r, sz))
                    r += sz
                for (rg, sz) in groups:
                    N = sz * 96
                    zps = ppz.tile([128, 512], F32, tag="zps")
                    prev_dst = None
                    for k in range(NCH + 1):
                        if k < NCH:
                            ps = pps.tile([128, 512], F32, tag="ps")
                            nc.tensor.matmul(ps[:, :N], kA[:, 128 * k:128 * (k + 1)],
                                             qA[:, rg * 96: rg * 96 + N],
                                             start=True, stop=False)
                        if prev_dst is not None:
                            # Z accumulation, pre-broadcast to all 128 partitions;
                            # placed between the sA/sB pair so the pair pipelines
                            # even when no deconv filler is available (stripe 0)
                            nc.tensor.matmul(zps[:, :N], o128[:], prev_dst,
                                             start=(k == 1), stop=(k == NCH))
                        if k == NCH:
                            break
                        nc.tensor.matmul(ps[:, :N], kB[:, 128 * k:128 * (k + 1)],
                                         qB[:, rg * 96: rg * 96 + N],
                                         start=False, stop=True)
                        # fill PE with previous-stripe deconv while ACT exps
                        emit_deconv_slice(9)
                        if k == 3 and deferred is not None:
                            deferred()
                            deferred = None
                        # exp(s) from psum -> att rows (scale folded into kpT)
                        dst = chunk_view(h, k)[:, rg:rg + sz, 1:97]
                        nc.scalar.activation(dst, ps[:, :N].rearrange("p (r c) -> p r c", c=96),
                                             AF.Exp)
                        prev_dst = dst

                    def tail(h=h, rg=rg, sz=sz, N=N, zps=zps):
                        # normalize: 1/Z straight to bf16, then scale att
                        bsb = prz.tile([128, 512], BF16, tag="bsb")
                        with nc.allow_low_precision(reason="1/Z in bf16 scales att"):
                            nc.vector.reciprocal(bsb[:, :N], zps[:, :N])
                        for k in range(NCH):
                            a_ap = chunk_view(h, k)[:, rg:rg + sz, 1:97]
                            nc.vector.tensor_mul(a_ap, a_ap,
                                                 bsb[:, :N].rearrange("p (r c) -> p r c", c=96))
                    deferred = tail

                # queue this stripe's deconv groups (run during next stripe)
                g0 = 0
                while g0 < ST_A:
                    G = min(5, ST_A - g0)
                    deconv_queue.append((h, a0 + g0, g0, G))
                    g0 += G
                # drain any unfinished pending deconv before stripe ends?
                # no — let it continue into the next stripe's blocks.

            # flush the last normalize tail, then drain remaining deconv groups
            if deferred is not None:
                deferred()
                deferred = None
            emit_deconv_slice(10 ** 9)
    nc.compile()
    return nc


def _prelu(z, a):
    return np.where(z >= 0, z, a * z)


def _host_prep(x, wa, ba, aa, w1, b1, a1, w2, b2, a2):
    """Per-sample gather prep (numpy, validated vs reference)."""
    f32 = np.float32
    per_core = []
    waT_aug = (np.concatenate([wa.T, ba[None, :]], 0) / 6.0).astype(f32)
    w1T_aug = np.concatenate([w1.T, b1[None, :]], 0).astype(f32)
    w2T_aug = np.concatenate([w2.T / 4.0, b2[None, :]], 0).astype(f32)
    aav, a1v, a2v = float(aa[0]), float(a1[0]), float(a2[0])
    for s in range(B):
        xs = np.asarray(x[s], f32)
        xq_aug = np.concatenate([xs.reshape(64, -1), np.ones((1, H * W), f32)], 0)
        asmT = _prelu(xq_aug.T @ waT_aug, aav)
        qT = _prelu(xq_aug.T @ w1T_aug, a1v)
        x3 = xs.reshape(64, 96, 96)
        t1 = x3[:, :, 0::2] + x3[:, :, 1::2]
        xd = t1[:, 0::2, :] + t1[:, 1::2, :]
        xd_aug = np.concatenate([xd.reshape(64, -1), np.ones((1, 48 * 48), f32)], 0)
        kfT = _prelu(xd_aug.T @ w2T_aug, a2v)

        kf = kfT.T.reshape(Cr, 48, 48)
        kpT = np.zeros((144, L), f32)
        for t, (dy, dx) in enumerate([(a, b) for a in range(3) for b in range(3)]):
            ly_lo, ly_hi = max(0, 1 - dy), min(48, 49 - dy)
            lx_lo, lx_hi = max(0, 1 - dx), min(48, 49 - dx)
            blk = kf[:, ly_lo + dy - 1:ly_hi + dy - 1, lx_lo + dx - 1:lx_hi + dx - 1]
            dst = kpT[16 * t:16 * t + 16].reshape(Cr, 48, 48)
            dst[:, ly_lo:ly_hi, lx_lo:lx_hi] = blk
        nrm = np.sqrt((kpT ** 2).sum(0))
        rnorm10 = (10.0 / np.maximum(nrm, 1e-4)).astype(f32)
        # fold the softmax scale / norm into kpT: scores psum = 10*s/norm
        kpT = kpT * rnorm10[None, :]

        q3 = qT.T.reshape(Cr, 96, 96)
        q_col = np.zeros((144, 96, 96), f32)
        for t, (dy, dx) in enumerate([(a, b) for a in range(3) for b in range(3)]):
            y_lo, y_hi = max(0, 1 - dy), min(96, 97 - dy)
            x_lo, x_hi = max(0, 1 - dx), min(96, 97 - dx)
            q_col[16 * t:16 * t + 16, y_lo:y_hi, x_lo:x_hi] = \
                q3[:, y_lo + dy - 1:y_hi + dy - 1, x_lo + dx - 1:x_hi + dx - 1]
        q_col = q_col.reshape(144, H * W)

        asm3 = asmT.T.reshape(64, 96, 96)
        for py in (0, 1):
            ap_t = np.zeros((3, 3, L, 128), f32)
            for n in range(3):
                u = py + 2 * n
                for m in range(3):
                    for half, v in ((0, 2 * m), (1, 2 * m + 1)):
                        ly_lo = max(0, (3 - u) // 2)
                        ly_hi = min(48, (99 - u) // 2)
                        lx_lo = max(0, (3 - v) // 2)
                        lx_hi = min(48, (97 - v) // 2 + 1)
                        Y0, X0 = 2 * ly_lo + u - 2, 2 * lx_lo + v - 2
                        blk = asm3[:, Y0:Y0 + 2 * (ly_hi - ly_lo):2,
                                   X0:X0 + 2 * (lx_hi - lx_lo):2]
                        dst = ap_t[n, m, :, 64 * half:64 * half + 64].reshape(48, 48, 64)
                        dst[ly_lo:ly_hi, lx_lo:lx_hi, :] = blk.transpose(1, 2, 0)
            # device ap layout: [p(128), k(18), nm(9), c(128)] for one-DMA load
            ap2 = np.ascontiguousarray(
                ap_t.reshape(9, NCH, 128, 128).transpose(2, 1, 0, 3)
            ).reshape(128, NCH * 9 * 128).astype(ml_dtypes.bfloat16)
            per_core.append({
                "qcolA": np.ascontiguousarray(q_col[:72]).astype(np.float16),
                "qcolB": np.ascontiguousarray(q_col[72:144]).astype(np.float16),
                "kpTA": np.ascontiguousarray(kpT[:72]).astype(np.float16),
                "kpTB": np.ascontiguousarray(kpT[72:144]).astype(np.float16),
                "ap": ap2,
            })
    return per_core


def kernel(x, wa, ba, aa, w1, b1, a1, w2, b2, a2):
    global last_exec_time_ns
    if "nc" not in _cache:
        _cache["nc"] = _build_program()
    nc = _cache["nc"]
    in_maps = _host_prep(np.asarray(x, np.float32), np.asarray(wa), np.asarray(ba),
                         np.asarray(aa), np.asarray(w1), np.asarray(b1),
                         np.asarray(a1), np.asarray(w2), np.asarray(b2),
                         np.asarray(a2))
    import os
    trace = bool(int(os.environ.get("KERNEL_TRACE", "0")))
    res = run_bass_kernel_spmd(nc, in_maps, core_ids=list(range(8)), trace=trace)
    last_exec_time_ns = res.exec_time_ns
    out = np.zeros((B, C, 192, 192), np.float32)
    for core in range(8):
        s, py = core // 2, core % 2
        r = res.results[core]["oh"].reshape(64, 96, 2, 96)
        out[s, :, py::2, 0::2] = r[:, :, 0, :]
        out[s, :, py::2, 1::2] = r[:, :, 1, :]
    return out
